# revision 1
# baseline (speedup 1.0000x reference)
"""Trainium2 Bass kernel for nn_CANE: data-parallel over batch on 8 NeuronCores.

Strategy: shard the batch (512 -> 64 items/core). Embedding tables (bf16,
rows padded to 128 elems = 256B) are replicated to every core's DRAM and
gathered on-device via transpose-mode dma_gather (text vocab split into two
<=32768-row halves to fit int16 indices; misses redirect to an all-zero row 0
and the two half-gathers are summed). All matmuls run in bf16 with fp32 PSUM
accumulation. Per-core scalar partial losses are summed on the host.
"""

import numpy as np
import ml_dtypes

import concourse.bass as bass
import concourse.bacc as bacc
import concourse.mybir as mybir
from concourse.tile import TileContext
from concourse import bass_utils

bf16 = ml_dtypes.bfloat16
F32 = mybir.dt.float32
BF = mybir.dt.bfloat16
I16 = mybir.dt.int16

B, NCORES = 512, 8
BL = B // NCORES            # 64 items per core
L, LM = 300, 299
E, C, V, NN = 100, 100, 50000, 100000
EP = 128                    # padded embedding row (256B in bf16)
NTOK = BL * L               # 19200 text tokens per tensor per core
TCH = 3200                  # gather chunk (25*128)
NCH = NTOK // TCH
HALF = 32767                # vocab ids < HALF go to the low table
NIDX = 256                  # node gather size (192 used, padded)
AF = mybir.ActivationFunctionType
ALU = mybir.AluOpType
AXL = mybir.AxisListType

# text tables: lo rows = 1 + HALF, hi rows = 1 + (V - HALF)
TLO_R, THI_R = HALF + 1, V - HALF + 1
# node tables: 4 splits of HALF ids each
NSPL = [(0, HALF), (HALF, 2 * HALF), (2 * HALF, 3 * HALF), (3 * HALF, NN)]
NTAB_R = [hi - lo + 1 for lo, hi in NSPL]

L_CK = [(0, 128), (128, 256), (256, 299)]   # l-chunks (the last is 43 wide)


def _wrap_idx(flat):
    """int16 flat index list -> [128, n/16] wrapped (i%16, i//16), x8 replicated."""
    n = flat.shape[0]
    assert n % 16 == 0
    w = flat.reshape(n // 16, 16).T.astype(np.int16)      # [16, n/16]
    return np.tile(w, (8, 1))                              # [128, n/16]


def _pad_rows(tab_f32):
    out = np.zeros((tab_f32.shape[0] + 1, EP), dtype=bf16)
    out[1:, :E] = tab_f32.astype(bf16)
    return out


def build_bass():
    nc = bacc.Bacc("TRN2", target_bir_lowering=False, debug=False)

    tlo = nc.dram_tensor("tlo", [TLO_R, EP], BF, kind="ExternalInput")
    thi = nc.dram_tensor("thi", [THI_R, EP], BF, kind="ExternalInput")
    ntab = [nc.dram_tensor(f"ntab{k}", [NTAB_R[k], EP], BF, kind="ExternalInput")
            for k in range(4)]
    tidx = nc.dram_tensor("tidx", [6, 128, NTOK // 16], I16, kind="ExternalInput")
    nidx = nc.dram_tensor("nidx", [4, 128, NIDX // 16], I16, kind="ExternalInput")
    w0td = nc.dram_tensor("w0td", [EP, C], BF, kind="ExternalInput")
    w1td = nc.dram_tensor("w1td", [EP, C], BF, kind="ExternalInput")
    rmatd = nc.dram_tensor("rmatd", [C, C], BF, kind="ExternalInput")
    biasd = nc.dram_tensor("biasd", [C, 1], F32, kind="ExternalInput")
    onesd = nc.dram_tensor("onesd", [128, 128], BF, kind="ExternalInput")  # all-ones
    identd = nc.dram_tensor("identd", [128, 128], BF, kind="ExternalInput")
    lossd = nc.dram_tensor("loss_out", [1, 1], F32, kind="ExternalOutput")

    with TileContext(nc) as tc:
        _emit(nc, tc, tlo, thi, ntab, tidx, nidx, w0td, w1td, rmatd, biasd,
              onesd, identd, lossd)
    nc.compile()  # Bacc: split multi-waits, insert library/act-table loads, lower ISA
    return nc


def _emit(nc, tc, tlo, thi, ntab, tidx, nidx, w0td, w1td, rmatd, biasd,
          onesd, identd, lossd):
    import contextlib
    ctx = contextlib.ExitStack()
    with ctx:
        const_p = ctx.enter_context(tc.tile_pool(name="const", bufs=1))
        txt_p = ctx.enter_context(tc.tile_pool(name="txt", bufs=1))
        raw_p = ctx.enter_context(tc.tile_pool(name="raw", bufs=2))
        work_p = ctx.enter_context(tc.tile_pool(name="work", bufs=3))
        coll_p = ctx.enter_context(tc.tile_pool(name="coll", bufs=1))
        bigps_p = ctx.enter_context(tc.tile_pool(name="bigps", bufs=1, space="PSUM"))
        smps_p = ctx.enter_context(tc.tile_pool(name="smps", bufs=2, space="PSUM"))

        # ---- constants into SBUF ----
        w0t = const_p.tile([EP, C], BF, name="w0t")
        w1t = const_p.tile([EP, C], BF, name="w1t")
        rmat = const_p.tile([C, C], BF, name="rmat")
        biasb = const_p.tile([C, 1], F32, name="biasb")
        onesb = const_p.tile([128, 128], BF, name="onesb")
        identb = const_p.tile([128, 128], BF, name="identb")
        nc.sync.dma_start(out=w0t[:, :], in_=w0td.ap())
        nc.sync.dma_start(out=w1t[:, :], in_=w1td.ap())
        nc.sync.dma_start(out=rmat[:, :], in_=rmatd.ap())
        nc.sync.dma_start(out=biasb[:, :], in_=biasd.ap())
        nc.sync.dma_start(out=onesb[:, :], in_=onesd.ap())
        nc.sync.dma_start(out=identb[:, :], in_=identd.ap())

        # ---- index tiles ----
        tix = const_p.tile([128, 6 * (NTOK // 16)], I16, name="tix")
        nix = const_p.tile([128, 4 * (NIDX // 16)], I16, name="nix")
        for t in range(6):
            nc.sync.dma_start(out=tix[:, t * (NTOK // 16):(t + 1) * (NTOK // 16)],
                              in_=tidx.ap()[t])
        for k in range(4):
            nc.sync.dma_start(out=nix[:, k * (NIDX // 16):(k + 1) * (NIDX // 16)],
                              in_=nidx.ap()[k])

        # ---- node gather: 4 splits summed; cols 3b+{0,1,2} = nA,nB,nN ----
        node_sb = coll_p.tile([128, NIDX], BF, name="node_sb")
        nraws = []
        for k in range(4):
            nraw = raw_p.tile([128, 1, NIDX], BF, name=f"nraw{k}", tag=f"nraw{k % 2}")
            nc.gpsimd.dma_gather(
                out_ap=nraw[:, :, :], in_ap=ntab[k].ap(),
                idxs_ap=nix[:, k * (NIDX // 16):(k + 1) * (NIDX // 16)],
                num_idxs=NIDX, num_idxs_reg=NIDX, elem_size=EP, transpose=True)
            nraws.append(nraw)
        # one DMA-wait per DVE op (multi-wait TT structs fail walrus codegen)
        nc.vector.tensor_copy(node_sb[:, :], nraws[0][:, 0, :])
        for k in (1, 2, 3):
            nc.vector.tensor_add(node_sb[:, :], node_sb[:, :], nraws[k][:, 0, :])

        # ---- text gathers: per tensor, 2 halves x NCH chunks, summed ----
        txts = []
        for t, tname in enumerate(("A", "B", "N")):
            txt = txt_p.tile([128, NTOK], BF, name=f"txt{tname}")
            txts.append(txt)
            for c in range(NCH):
                rhi = raw_p.tile([128, 1, TCH], BF, name=f"rhi{t}_{c}", tag="rhi")
                i0 = (2 * t) * (NTOK // 16) + c * (TCH // 16)
                i1 = (2 * t + 1) * (NTOK // 16) + c * (TCH // 16)
                dst = txt[:, c * TCH:(c + 1) * TCH]
                dst3 = txt.rearrange("p (k n) -> p k n", n=TCH)[:, c:c + 1, :]
                nc.gpsimd.dma_gather(
                    out_ap=dst3, in_ap=tlo.ap(),
                    idxs_ap=tix[:, i0:i0 + TCH // 16],
                    num_idxs=TCH, num_idxs_reg=TCH, elem_size=EP, transpose=True,
                    single_packet=False)
                nc.gpsimd.dma_gather(
                    out_ap=rhi[:, :, :], in_ap=thi.ap(),
                    idxs_ap=tix[:, i1:i1 + TCH // 16],
                    num_idxs=TCH, num_idxs_reg=TCH, elem_size=EP, transpose=True,
                    single_packet=False)
                nc.vector.tensor_add(dst, dst, rhi[:, 0, :])

        # ---- per-core collectors ----
        convcols = coll_p.tile([101, 3 * BL], F32, name="convcols")
        rawdots = coll_p.tile([1, 8 * BL], F32, name="rawdots")

        # ---- per-item pipeline ----
        for b in range(BL):
            cb = b * L
            bigp = bigps_p.tile([128, 6, 512], F32, name=f"bigp{b}", tag="bigp")
            hmrp = smps_p.tile([128, 512], F32, name=f"hmrp{b}", tag="smps")
            rowp = smps_p.tile([128, 512], F32, name=f"rowp{b}", tag="smps")
            bcpA = smps_p.tile([128, 512], F32, name=f"bcpA{b}", tag="smps")
            bcpB = smps_p.tile([128, 512], F32, name=f"bcpB{b}", tag="smps")
            bcpN = smps_p.tile([128, 512], F32, name=f"bcpN{b}", tag="smps")
            hx = work_p.tile([128, 3, LM], BF, name=f"hx{b}", tag="hx")
            hmrq = work_p.tile([C, 384], BF, name=f"hmrq{b}", tag="hmrq")
            t1 = work_p.tile([128, 6, LM], BF, name=f"t1_{b}", tag="t1")
            scr = work_p.tile([101, LM], BF, name=f"scr{b}", tag="scr")
            wraw = work_p.tile([128, 3], F32, name=f"wraw{b}", tag="wraw")
            eac = work_p.tile([128, 3], BF, name=f"eac{b}", tag="eac")
            erow = work_p.tile([1, 3, LM], BF, name=f"erow{b}", tag="erow")

            # conv: psum[0:100, t, 0:299] = w0t.T@txt[:,cb:cb+299] + w1t.T@(shift)
            for t in range(3):
                nc.tensor.matmul(bigp[0:C, t, 0:LM], w0t[:, :],
                                 txts[t][:, cb:cb + LM], start=True, stop=False)
            for t in range(3):
                nc.tensor.matmul(bigp[0:C, t, 0:LM], w1t[:, :],
                                 txts[t][:, cb + 1:cb + L], start=False, stop=True)
            # ones rows 96:128 first; conv-tanh then overwrites 96:100 with real
            # values, leaving rows 100+ = 1.0 (engine APs must start at 0/32/64/96)
            nc.vector.memset(hx[96:128, :, :], 1.0)
            nc.scalar.activation(hx[0:C, :, :], bigp[0:C, 0:3, 0:LM], AF.Tanh,
                                 bias=biasb[:, :], scale=1.0)

            # hmr: psum = rmat.T @ hAT ; copy to bf16, zero-pad cols 299:384
            nc.tensor.matmul(hmrp[0:C, 0:LM], rmat[:, :], hx[0:C, 0, :],
                             start=True, stop=True)
            nc.vector.tensor_copy(hmrq[:, 0:LM], hmrp[0:C, 0:LM])
            nc.vector.memset(hmrq[:, LM:384], 0.0)

            # att: slots 0-2 = att1 (rhs hB), slots 3-5 = att3 (rhs hN)
            for ck in range(3):
                lhs = hmrq[:, ck * 128:(ck + 1) * 128]
                nc.tensor.matmul(bigp[:, ck, 0:LM], lhs, hx[0:C, 1, :],
                                 start=True, stop=True)
                nc.tensor.matmul(bigp[:, 3 + ck, 0:LM], lhs, hx[0:C, 2, :],
                                 start=True, stop=True)
            nc.scalar.activation(t1[:, :, :], bigp[:, 0:6, 0:LM], AF.Tanh)

            # wA: free-dim reduce of att1 chunks -> [128,3]; exp -> bf16 cols
            nc.vector.tensor_reduce(wraw[:, :], t1[:, 0:3, :], axis=AXL.X, op=ALU.add)
            nc.scalar.activation(eac[:, :], wraw[:, :], AF.Exp, scale=1.0 / LM)

            # wB / wNEG: column sums via ones-matmuls (accumulate over chunks)
            for ck, (l0, l1) in enumerate(L_CK):
                w = l1 - l0
                nc.tensor.matmul(rowp[0:1, 0:LM], onesb[0:w, 0:1],
                                 t1[0:w, ck, :], start=(ck == 0), stop=(ck == 2))
                nc.tensor.matmul(hmrp[0:1, 0:LM], onesb[0:w, 0:1],
                                 t1[0:w, 3 + ck, :], start=(ck == 0), stop=(ck == 2))
            # rows: eB, eN from psum; eA via transpose of eac columns
            nc.scalar.activation(erow[:, 1, :], rowp[0:1, 0:LM], AF.Exp,
                                 scale=1.0 / LM)
            nc.scalar.activation(erow[:, 2, :], hmrp[0:1, 0:LM], AF.Exp,
                                 scale=1.0 / LM)
            for ck, (l0, l1) in enumerate(L_CK):
                w = l1 - l0
                nc.tensor.matmul(rowp[0:1, l0:l1], eac[0:w, ck:ck + 1],
                                 identb[0:w, 0:w], start=True, stop=True)
            nc.vector.tensor_copy(erow[:, 0, :], rowp[0:1, 0:LM])

            # broadcast rows to 101 partitions (outer product with ones col)
            bcps = (bcpA, bcpB, bcpN)
            for t in range(3):
                nc.tensor.matmul(bcps[t][0:C + 1, 0:LM], onesb[0:1, 0:C + 1],
                                 erow[:, t, :], start=True, stop=True)
            # conv vectors + sums: reduce of hx_ext * bc  (row 100 = ones -> sX)
            # (tensor_tensor_reduce hard-crashes this runtime; use mult+reduce)
            for t in range(3):
                nc.vector.tensor_tensor(out=scr[:, :], in0=hx[0:C + 1, t, :],
                                        in1=bcps[t][0:C + 1, 0:LM], op=ALU.mult)
                nc.vector.tensor_reduce(convcols[:, 3 * b + t:3 * b + t + 1],
                                        scr[:, :], axis=AXL.X, op=ALU.add)

        # ---- dots phase ----
        ccb = coll_p.tile([101, 3 * BL], BF, name="ccb")
        nc.vector.tensor_copy(ccb[:, :], convcols[:, :])
        for b in range(BL):
            dps = smps_p.tile([128, 512], F32, name=f"dps{b}", tag="smps")
            cA = ccb[0:C, 3 * b:3 * b + 1]
            cBN = ccb[0:C, 3 * b + 1:3 * b + 3]
            nA = node_sb[0:C, 3 * b:3 * b + 1]
            nBc = node_sb[0:C, 3 * b + 1:3 * b + 2]
            nBN = node_sb[0:C, 3 * b + 1:3 * b + 3]
            nc.tensor.matmul(dps[0:1, 0:2], cA, cBN, start=True, stop=True)
            nc.tensor.matmul(dps[0:1, 2:3], cA, nBc, start=True, stop=True)
            nc.tensor.matmul(dps[0:1, 3:5], nA, nBN, start=True, stop=True)
            nc.tensor.matmul(dps[0:1, 5:7], nA, cBN, start=True, stop=True)
            nc.tensor.matmul(dps[0:1, 7:8], nBc, ccb[0:C, 3 * b + 2:3 * b + 3],
                             start=True, stop=True)
            nc.vector.tensor_copy(rawdots[:, 8 * b:8 * b + 8], dps[0:1, 0:8])

        # ---- finals (row layout, vectorized over the 64 items) ----
        srow = coll_p.tile([1, 3 * BL], F32, name="srow")
        nc.sync.dma_start(out=srow[:, :], in_=convcols[C:C + 1, :])
        rr = coll_p.tile([1, 3 * BL], F32, name="rr")
        nc.vector.reciprocal(rr[:, :], srow[:, :])
        xs = coll_p.tile([1, 8 * BL], F32, name="xs")
        tmpa = coll_p.tile([1, BL], F32, name="tmpa")
        tmpb = coll_p.tile([1, BL], F32, name="tmpb")

        def dslice(k):
            return rawdots[0:1, k::8]

        def xslice(k):
            return xs[0:1, k::8]

        def rA():
            return rr[0:1, 0::3]

        def rB():
            return rr[0:1, 1::3]

        def rN():
            return rr[0:1, 2::3]

        # rawdots col order: [s1, s2, s7, s3, s4, s5, s6, s8]
        nc.vector.tensor_mul(tmpa[:, :], dslice(0), rA())
        nc.vector.tensor_mul(xslice(0), tmpa[:, :], rB())          # +s1 rA rB
        nc.vector.tensor_mul(tmpa[:, :], dslice(1), rA())
        nc.vector.tensor_mul(tmpb[:, :], tmpa[:, :], rN())
        nc.vector.tensor_scalar_mul(xslice(1), tmpb[:, :], -1.0)   # -s2 rA rN
        nc.vector.tensor_copy(xslice(2), dslice(3))                # +s3
        nc.vector.tensor_scalar_mul(xslice(3), dslice(4), -1.0)    # -s4
        nc.vector.tensor_mul(xslice(4), dslice(5), rB())           # +s5 rB
        nc.vector.tensor_mul(tmpa[:, :], dslice(6), rN())
        nc.vector.tensor_scalar_mul(xslice(5), tmpa[:, :], -1.0)   # -s6 rN
        nc.vector.tensor_mul(xslice(6), dslice(2), rA())           # +s7 rA
        nc.vector.tensor_mul(tmpa[:, :], dslice(7), rN())
        nc.vector.tensor_scalar_mul(xslice(7), tmpa[:, :], -1.0)   # -s8 rN

        sg = coll_p.tile([1, 8 * BL], F32, name="sg")
        pl = coll_p.tile([1, 8 * BL], F32, name="pl")
        nc.scalar.activation(sg[:, :], xs[:, :], AF.Sigmoid)
        nc.vector.tensor_scalar_add(sg[:, :], sg[:, :], 0.001)
        nc.scalar.activation(pl[:, :], sg[:, :], AF.Ln)

        def pslice(k):
            return pl[0:1, k::8]

        acc1 = coll_p.tile([1, BL], F32, name="acc1")
        acc3 = coll_p.tile([1, BL], F32, name="acc3")
        nc.vector.tensor_add(acc1[:, :], pslice(0), pslice(1))
        nc.vector.tensor_add(acc3[:, :], pslice(2), pslice(3))
        for k in (4, 5, 6, 7):
            nc.vector.tensor_add(acc3[:, :], acc3[:, :], pslice(k))
        nc.vector.tensor_scalar_mul(acc3[:, :], acc3[:, :], 0.3)
        nc.vector.tensor_add(acc1[:, :], acc1[:, :], acc3[:, :])
        lsum = coll_p.tile([1, 1], F32, name="lsum")
        nc.vector.tensor_reduce(lsum[:, :], acc1[:, :], axis=AXL.X, op=ALU.add)
        nc.vector.tensor_scalar_mul(lsum[:, :], lsum[:, :], -1.0)
        nc.sync.dma_start(out=lossd.ap(), in_=lsum[:, :])


# ----------------------------------------------------------------------------
# host side
# ----------------------------------------------------------------------------

def _text_idx_arrays(T):
    """T: [BL, L] int -> (lo, hi) wrapped int16 [128, NTOK/16]."""
    flat = T.reshape(-1).astype(np.int64)
    lo = np.where(flat < HALF, flat + 1, 0).astype(np.int16)
    hi = np.where(flat >= HALF, flat - HALF + 1, 0).astype(np.int16)
    return _wrap_idx(lo), _wrap_idx(hi)


def _node_idx_arrays(Na, Nb, Nn):
    inter = np.stack([Na, Nb, Nn], axis=1).reshape(-1).astype(np.int64)  # [192]
    inter = np.concatenate([inter, np.full(NIDX - inter.shape[0], -10, np.int64)])
    outs = []
    for lo, hi in NSPL:
        sel = (inter >= lo) & (inter < hi)
        ids = np.where(sel, inter - lo + 1, 0).astype(np.int16)
        outs.append(_wrap_idx(ids))
    return outs


_CACHED_NC = None


def kernel(**inputs):
    global _CACHED_NC
    text_emb = np.asarray(inputs["text_emb"], np.float32)
    node_emb = np.asarray(inputs["node_emb"], np.float32)
    conv_w = np.asarray(inputs["conv_w"], np.float32)
    conv_b = np.asarray(inputs["conv_b"], np.float32)
    rmat = np.asarray(inputs["rand_matrix"], np.float32)

    tlo_a = _pad_rows(text_emb[:HALF])                   # [32768, 128]
    thi_a = _pad_rows(text_emb[HALF:])
    ntab_a = [_pad_rows(node_emb[lo:hi]) for lo, hi in NSPL]
    w0t_a = np.zeros((EP, C), bf16); w0t_a[:E] = conv_w[:, 0, 0, :].T.astype(bf16)
    w1t_a = np.zeros((EP, C), bf16); w1t_a[:E] = conv_w[:, 0, 1, :].T.astype(bf16)
    rmat_a = rmat.astype(bf16)
    bias_a = conv_b.reshape(C, 1).astype(np.float32)
    ones_a = np.ones((128, 128), bf16)
    ident_a = np.eye(128, dtype=bf16)

    if _CACHED_NC is None:
        _CACHED_NC = build_bass()
    nc = _CACHED_NC

    in_maps = []
    for core in range(NCORES):
        sl = slice(core * BL, (core + 1) * BL)
        tA = np.asarray(inputs["Text_a"])[sl]
        tB = np.asarray(inputs["Text_b"])[sl]
        tN = np.asarray(inputs["Text_neg"])[sl]
        nA = np.asarray(inputs["Node_a"])[sl]
        nB = np.asarray(inputs["Node_b"])[sl]
        nN = np.asarray(inputs["Node_neg"])[sl]
        tidx_a = np.stack([w for T in (tA, tB, tN) for w in _text_idx_arrays(T)])
        nidx_a = np.stack(_node_idx_arrays(nA, nB, nN))
        m = {
            "tlo": tlo_a, "thi": thi_a,
            "tidx": tidx_a, "nidx": nidx_a,
            "w0td": w0t_a, "w1td": w1t_a, "rmatd": rmat_a, "biasd": bias_a,
            "onesd": ones_a, "identd": ident_a,
        }
        for k in range(4):
            m[f"ntab{k}"] = ntab_a[k]
        in_maps.append(m)

    res = bass_utils.run_bass_kernel_spmd(nc, in_maps, core_ids=list(range(NCORES)))
    parts = [float(r["loss_out"][0, 0]) for r in res.results]
    return np.float32(np.sum(parts, dtype=np.float64))



# revision 10
# speedup vs baseline: 1.9112x; 1.9112x over previous
"""Trainium2 Bass kernel for nn_CANE: data-parallel over batch on 8 NeuronCores.

Strategy: shard the batch (512 -> 64 items/core). Embedding tables (bf16,
rows padded to 128 elems = 256B) are replicated to every core's DRAM and
gathered on-device via transpose-mode dma_gather (text vocab split into two
<=32768-row halves to fit int16 indices; misses redirect to an all-zero row 0
and the two half-gathers are summed). All matmuls run in bf16 with fp32 PSUM
accumulation. Per-core scalar partial losses are summed on the host.

Pipeline layout (v2): separate PSUM tiles per stage (conv 3 banks, att 3
banks, transpose 1 bank, small 1 bank) so consecutive items overlap across
engines. Attention-weight reductions run on the PE (column sums as N=1
matmuls, conv vectors as transposed-h matvecs); PSUM->SBUF copies run on
the Pool engine; the ones rows / zero pads are memset once outside the loop.
"""

import numpy as np
import ml_dtypes

import concourse.bass as bass
import concourse.bacc as bacc
import concourse.mybir as mybir
from concourse.tile import TileContext
from concourse import bass_utils

bf16 = ml_dtypes.bfloat16
F32 = mybir.dt.float32
BF = mybir.dt.bfloat16
I16 = mybir.dt.int16

B, NCORES = 512, 8
BL = B // NCORES            # 64 items per core
L, LM = 300, 299
E, C, V, NN = 100, 100, 50000, 100000
EP = 128                    # padded embedding row (256B in bf16)
NTOK = BL * L               # 19200 text tokens per tensor per core
TCH = 3200                  # gather chunk (25*128)
NCH = NTOK // TCH
HALF = 32767                # vocab ids < HALF go to the low table
NIDX = 256                  # node gather size (192 used, padded)
AF = mybir.ActivationFunctionType
ALU = mybir.AluOpType
AXL = mybir.AxisListType

# text tables: lo rows = 1 + HALF, hi rows = 1 + (V - HALF)
TLO_R, THI_R = HALF + 1, V - HALF + 1
# node tables: 4 splits of HALF ids each
NSPL = [(0, HALF), (HALF, 2 * HALF), (2 * HALF, 3 * HALF), (3 * HALF, NN)]
NTAB_R = [hi - lo + 1 for lo, hi in NSPL]

L_CK = [(0, 128), (128, 256), (256, 299)]   # l/m-chunks (the last is 43 wide)


def _wrap_idx(flat):
    """int16 flat index list -> [128, n/16] wrapped (i%16, i//16), x8 replicated."""
    n = flat.shape[0]
    assert n % 16 == 0
    w = flat.reshape(n // 16, 16).T.astype(np.int16)      # [16, n/16]
    return np.tile(w, (8, 1))                              # [128, n/16]


def _pad_rows(tab_f32):
    out = np.zeros((tab_f32.shape[0] + 1, EP), dtype=bf16)
    out[1:, :E] = tab_f32.astype(bf16)
    return out


def build_bass():
    nc = bacc.Bacc("TRN2", target_bir_lowering=False, debug=False)

    tlo = nc.dram_tensor("tlo", [TLO_R, EP], BF, kind="ExternalInput")
    thi = nc.dram_tensor("thi", [THI_R, EP], BF, kind="ExternalInput")
    ntab = [nc.dram_tensor(f"ntab{k}", [NTAB_R[k], EP], BF, kind="ExternalInput")
            for k in range(4)]
    tidx = nc.dram_tensor("tidx", [6, 128, NTOK // 16], I16, kind="ExternalInput")
    nidx = nc.dram_tensor("nidx", [4, 128, NIDX // 16], I16, kind="ExternalInput")
    w0td = nc.dram_tensor("w0td", [EP, C], BF, kind="ExternalInput")
    w1td = nc.dram_tensor("w1td", [EP, C], BF, kind="ExternalInput")
    rmatd = nc.dram_tensor("rmatd", [C, C], BF, kind="ExternalInput")
    biasd = nc.dram_tensor("biasd", [C, 1], F32, kind="ExternalInput")
    onesd = nc.dram_tensor("onesd", [128, 128], BF, kind="ExternalInput")  # all-ones
    identd = nc.dram_tensor("identd", [128, 128], BF, kind="ExternalInput")
    lossd = nc.dram_tensor("loss_out", [1, 1], F32, kind="ExternalOutput")

    with TileContext(nc) as tc:
        _emit(nc, tc, tlo, thi, ntab, tidx, nidx, w0td, w1td, rmatd, biasd,
              onesd, identd, lossd)
    nc.compile()  # Bacc: split multi-waits, insert library/act-table loads, lower ISA
    return nc


def _emit(nc, tc, tlo, thi, ntab, tidx, nidx, w0td, w1td, rmatd, biasd,
          onesd, identd, lossd):
    import contextlib
    ctx = contextlib.ExitStack()
    with ctx:
        const_p = ctx.enter_context(tc.tile_pool(name="const", bufs=1))
        txt_p = ctx.enter_context(tc.tile_pool(name="txt", bufs=1))
        raw_p = ctx.enter_context(tc.tile_pool(name="raw", bufs=2))
        work_p = ctx.enter_context(tc.tile_pool(name="work", bufs=1))
        coll_p = ctx.enter_context(tc.tile_pool(name="coll", bufs=1))
        convps_p = ctx.enter_context(tc.tile_pool(name="convps", bufs=1, space="PSUM"))
        attps_p = ctx.enter_context(tc.tile_pool(name="attps", bufs=1, space="PSUM"))
        mixps_p = ctx.enter_context(tc.tile_pool(name="mixps", bufs=1, space="PSUM"))
        hmrps_p = ctx.enter_context(tc.tile_pool(name="hmrps", bufs=1, space="PSUM"))

        # ---- constants into SBUF ----
        w0t = const_p.tile([EP, C], BF, name="w0t")
        w1t = const_p.tile([EP, C], BF, name="w1t")
        rmat = const_p.tile([C, C], BF, name="rmat")
        biasb = const_p.tile([C, 1], F32, name="biasb")
        onesb = const_p.tile([128, 128], BF, name="onesb")
        identb = const_p.tile([128, 128], BF, name="identb")
        nc.sync.dma_start(out=w0t[:, :], in_=w0td.ap())
        nc.sync.dma_start(out=w1t[:, :], in_=w1td.ap())
        nc.sync.dma_start(out=rmat[:, :], in_=rmatd.ap())
        nc.sync.dma_start(out=biasb[:, :], in_=biasd.ap())
        nc.sync.dma_start(out=onesb[:, :], in_=onesd.ap())
        nc.sync.dma_start(out=identb[:, :], in_=identd.ap())

        # ---- index tiles ----
        tix = const_p.tile([128, 6 * (NTOK // 16)], I16, name="tix")
        nix = const_p.tile([128, 4 * (NIDX // 16)], I16, name="nix")
        for t in range(6):
            nc.sync.dma_start(out=tix[:, t * (NTOK // 16):(t + 1) * (NTOK // 16)],
                              in_=tidx.ap()[t])
        for k in range(4):
            nc.sync.dma_start(out=nix[:, k * (NIDX // 16):(k + 1) * (NIDX // 16)],
                              in_=nidx.ap()[k])

        # ---- node gather: 4 splits summed; cols 3b+{0,1,2} = nA,nB,nN ----
        # (gathers issued up front on Pool; the DVE sum happens after the
        # item loop so it doesn't block item work in the DVE stream)
        node_sb = coll_p.tile([128, NIDX], BF, name="node_sb")
        nraws = []
        for k in range(4):
            nraw = raw_p.tile([128, 1, NIDX], BF, name=f"nraw{k}", tag=f"nraw{k % 2}")
            nc.gpsimd.dma_gather(
                out_ap=nraw[:, :, :], in_ap=ntab[k].ap(),
                idxs_ap=nix[:, k * (NIDX // 16):(k + 1) * (NIDX // 16)],
                num_idxs=NIDX, num_idxs_reg=NIDX, elem_size=EP, transpose=True)
            nraws.append(nraw)

        txts = []
        for t, tname in enumerate(("A", "B", "N")):
            txt = txt_p.tile([128, NTOK], BF, name=f"txt{tname}")
            txts.append(txt)
        rhis = [[None] * 3 for _ in range(NCH)]

        def emit_gathers(c):
            for t in range(3):
                txt = txts[t]
                rhi = raw_p.tile([128, 1, TCH], BF, name=f"rhi{t}_{c}",
                                 tag=f"rhi{t}")
                rhis[c][t] = rhi
                i0 = (2 * t) * (NTOK // 16) + c * (TCH // 16)
                i1 = (2 * t + 1) * (NTOK // 16) + c * (TCH // 16)
                dst3 = txt.rearrange("p (k n) -> p k n", n=TCH)[:, c:c + 1, :]
                nc.gpsimd.dma_gather(
                    out_ap=dst3, in_ap=tlo.ap(),
                    idxs_ap=tix[:, i0:i0 + TCH // 16],
                    num_idxs=TCH, num_idxs_reg=TCH, elem_size=EP, transpose=True,
                    single_packet=False)
                nc.gpsimd.dma_gather(
                    out_ap=rhi[:, :, :], in_ap=thi.ap(),
                    idxs_ap=tix[:, i1:i1 + TCH // 16],
                    num_idxs=TCH, num_idxs_reg=TCH, elem_size=EP, transpose=True,
                    single_packet=False)

        def emit_adds(c):
            for t in range(3):
                dst = txts[t][:, c * TCH:(c + 1) * TCH]
                nc.vector.tensor_add(dst, dst, rhis[c][t][:, 0, :])

        # ---- per-core collectors ----
        convcols = coll_p.tile([101, 3 * BL], F32, name="convcols")
        rawdots = coll_p.tile([1, 8 * BL], F32, name="rawdots")

        # ---- rotating SBUF work tiles (manual rotation; ones/zero pads are
        #      set once and never overwritten inside the loop) ----
        NROT = 3
        hxs, hmqs, t11s, t13s, htxs, eacs, ecols, wraws = ([] for _ in range(8))
        for r in range(NROT):
            hx = work_p.tile([128, 3, LM], BF, name=f"hx{r}")
            nc.vector.memset(hx[96:128, :, :], 1.0)   # rows 100..127 stay 1.0
            hxs.append(hx)
            hmq = work_p.tile([C, 384], BF, name=f"hmq{r}")
            nc.vector.memset(hmq[:, LM:384], 0.0)
            hmqs.append(hmq)
            t11s.append(work_p.tile([128, 3, LM], BF, name=f"t11_{r}"))
            t13s.append(work_p.tile([128, 3, LM], BF, name=f"t13_{r}"))
            htxs.append(work_p.tile([128, 9, 104], BF, name=f"htx{r}"))
            eacs.append(work_p.tile([128, 3], BF, name=f"eac{r}"))
            ecols.append(work_p.tile([128, 6], BF, name=f"ecol{r}"))
            wraws.append(work_p.tile([128, 3], F32, name=f"wraw{r}"))

        # ---- static PSUM tiles (8 banks total); item reuse is serialized by
        #      bank-level WAR deps, which is exactly the pipeline rhythm ----
        convp = convps_p.tile([128, 3, 512], F32, name="convp")
        attp = attps_p.tile([128, 3, 512], F32, name="attp")
        # transpose slots padded to 104 elems (208B) for 4-byte PSUM alignment;
        # cvp shares this bank (its accesses sit between trx/htx in the chain)
        trxp = mixps_p.tile([128, 9, 104], BF, name="trxp")
        cvpt = mixps_p.tile([128, 8], F32, name="cvpt")
        cvps = (cvpt[:, 0:3], cvpt[:, 4:7])
        hmrb = hmrps_p.tile([128, 512], F32, name="hmrb")
        hmrp = hmrb[:, 0:LM]                  # [.., 299]
        ecolps = (hmrb[:, 304:310], hmrb[:, 312:318])     # rotate even/odd item
        # dots region: hmrb cols 320:384, 8 items x 8 dots

        # ---- 3-stage software pipeline over items ----
        # S0(b): conv mm+tanh, hmr mm+copy, att1 mm  (hoisted one item ahead)
        # SA(b): tanh1, att3, tanh3, trx, htx, wraw, eac
        # S2(b): colsum (tail of producing iter), ecol exp, matvec, collect
        def emit_s0(b):
            cb = b * L
            r = b % NROT
            hx, hmrq = hxs[r], hmqs[r]
            for t in range(3):
                nc.tensor.matmul(convp[0:C, t, 0:LM], w0t[:, :],
                                 txts[t][:, cb:cb + LM], start=True, stop=False)
            for t in range(3):
                nc.tensor.matmul(convp[0:C, t, 0:LM], w1t[:, :],
                                 txts[t][:, cb + 1:cb + L], start=False, stop=True)
            # conv-tanh writes rows 0:100; rows 100:127 stay 1.0 from the memset
            nc.scalar.activation(hx[0:C, :, :], convp[0:C, 0:3, 0:LM], AF.Tanh,
                                 bias=biasb[:, :], scale=1.0)
            # hmr: psum = rmat.T @ hA ; copy to bf16 (DVE — gpsimd has no PSUM port)
            nc.tensor.matmul(hmrp[0:C, 0:LM], rmat[:, :], hx[0:C, 0, :],
                             start=True, stop=True)
            nc.vector.tensor_copy(hmrq[:, 0:LM], hmrp[0:C, 0:LM])
            # att1 (rhs hB)
            for ck in range(3):
                lhs = hmrq[:, ck * 128:(ck + 1) * 128]
                nc.tensor.matmul(attp[:, ck, 0:LM], lhs, hx[0:C, 1, :],
                                 start=True, stop=True)

        def emit_colsum(b):
            r = b % NROT
            t11, t13 = t11s[r], t13s[r]
            ecolp = ecolps[b % 2]
            for j, t1x in enumerate((t11, t13)):
                for mk, (m0, m1) in enumerate(L_CK):
                    for lk, (l0, l1) in enumerate(L_CK):
                        wl = l1 - l0
                        nc.tensor.matmul(
                            ecolp[0:m1 - m0, 3 * j + mk:3 * j + mk + 1],
                            t1x[0:wl, lk, m0:m1], onesb[0:wl, 0:1],
                            start=(lk == 0), stop=(lk == 2))

        def emit_sa(b):
            r = b % NROT
            hx, hmrq = hxs[r], hmqs[r]
            t11, t13, htx = t11s[r], t13s[r], htxs[r]
            eac, wraw = eacs[r], wraws[r]
            nc.scalar.activation(t11[:, :, :], attp[:, 0:3, 0:LM], AF.Tanh)
            # att3 (rhs hN) reusing the att psum banks after tanh1 reads them
            for ck in range(3):
                lhs = hmrq[:, ck * 128:(ck + 1) * 128]
                nc.tensor.matmul(attp[:, ck, 0:LM], lhs, hx[0:C, 2, :],
                                 start=True, stop=True)
            # transposes: h_ext[0:101, t, l0:l1] -> trxp[l, 3t+ck, 0:101]
            for t in range(3):
                for ck, (l0, l1) in enumerate(L_CK):
                    w = l1 - l0
                    nc.tensor.transpose(trxp[0:w, 3 * t + ck, 0:101],
                                        hx[0:C + 1, t, l0:l1],
                                        identb[0:C + 1, 0:C + 1])
            nc.vector.tensor_reduce(wraw[:, :], t11[:, 0:3, :], axis=AXL.X,
                                    op=ALU.add)
            nc.scalar.activation(t13[:, :, :], attp[:, 0:3, 0:LM], AF.Tanh)
            nc.vector.tensor_copy(htx[:, :, :], trxp[:, :, :])
            nc.scalar.activation(eac[:, :], wraw[:, :], AF.Exp, scale=1.0 / LM)

        def emit_s2(b):
            """ecol exp -> conv-vector matvecs -> collect (colsum ran earlier)."""
            r = b % NROT
            htx = htxs[r]
            eac, ecol = eacs[r], ecols[r]
            ecolp = ecolps[b % 2]
            cvp = cvps[b % 2]
            nc.scalar.activation(ecol[:, :], ecolp[:, :], AF.Exp, scale=1.0 / LM)
            for t in range(3):
                for ck, (l0, l1) in enumerate(L_CK):
                    w = l1 - l0
                    if t == 0:
                        ecv = eac[0:w, ck:ck + 1]
                    else:
                        ecv = ecol[0:w, 3 * (t - 1) + ck:3 * (t - 1) + ck + 1]
                    nc.tensor.matmul(cvp[0:C + 1, t:t + 1],
                                     htx[0:w, 3 * t + ck, 0:101], ecv,
                                     start=(ck == 0), stop=(ck == 2))
            nc.vector.tensor_copy(convcols[:, 3 * b:3 * b + 3], cvp[0:C + 1, :])

        # ---- interleaved emission: chunk-c gathers stay ahead; iteration b
        #      emits S2(b-1), SA(b), then S0(b+1) + colsum(b) ----
        emit_gathers(0)
        emit_gathers(1)
        emit_adds(0)
        emit_s0(0)
        done_chunk = 1
        for b in range(BL):
            need = min(BL - 1, b + 1)  # S0(b+1) reads item b+1's text cols
            c_need = ((need + 1) * L - 1) // TCH
            while done_chunk < c_need + 1 and done_chunk < NCH:
                if done_chunk + 1 < NCH:
                    emit_gathers(done_chunk + 1)
                emit_adds(done_chunk)
                done_chunk += 1
            if b > 0:
                emit_s2(b - 1)
            emit_sa(b)
            if b + 1 < BL:
                emit_s0(b + 1)
            emit_colsum(b)
        emit_s2(BL - 1)

        # node embedding sum (deferred so the DVE stream stays clear)
        nc.vector.tensor_copy(node_sb[:, :], nraws[0][:, 0, :])
        for k in (1, 2, 3):
            nc.vector.tensor_add(node_sb[:, :], node_sb[:, :], nraws[k][:, 0, :])

        # ---- dots phase ----
        ccb = coll_p.tile([101, 3 * BL], BF, name="ccb")
        nc.vector.tensor_copy(ccb[:, :], convcols[:, :])
        for b in range(BL):
            g = b % 8
            dps = small[0:1, 336 + 8 * g:336 + 8 * g + 8]
            cA = ccb[0:C, 3 * b:3 * b + 1]
            cBN = ccb[0:C, 3 * b + 1:3 * b + 3]
            nA = node_sb[0:C, 3 * b:3 * b + 1]
            nBc = node_sb[0:C, 3 * b + 1:3 * b + 2]
            nBN = node_sb[0:C, 3 * b + 1:3 * b + 3]
            nc.tensor.matmul(dps[:, 0:2], cA, cBN, start=True, stop=True)
            nc.tensor.matmul(dps[:, 2:3], cA, nBc, start=True, stop=True)
            nc.tensor.matmul(dps[:, 3:5], nA, nBN, start=True, stop=True)
            nc.tensor.matmul(dps[:, 5:7], nA, cBN, start=True, stop=True)
            nc.tensor.matmul(dps[:, 7:8], nBc, ccb[0:C, 3 * b + 2:3 * b + 3],
                             start=True, stop=True)
            if g == 7:
                nc.vector.tensor_copy(rawdots[:, 8 * (b - 7):8 * (b + 1)],
                                      small[0:1, 320:320 + 64])

        # ---- finals (row layout, vectorized over the 64 items) ----
        srow = coll_p.tile([1, 3 * BL], F32, name="srow")
        nc.sync.dma_start(out=srow[:, :], in_=convcols[C:C + 1, :])
        rr = coll_p.tile([1, 3 * BL], F32, name="rr")
        nc.vector.reciprocal(rr[:, :], srow[:, :])
        xs = coll_p.tile([1, 8 * BL], F32, name="xs")
        tmpa = coll_p.tile([1, BL], F32, name="tmpa")
        tmpb = coll_p.tile([1, BL], F32, name="tmpb")

        def dslice(k):
            return rawdots[0:1, k::8]

        def xslice(k):
            return xs[0:1, k::8]

        def rA():
            return rr[0:1, 0::3]

        def rB():
            return rr[0:1, 1::3]

        def rN():
            return rr[0:1, 2::3]

        # rawdots col order: [s1, s2, s7, s3, s4, s5, s6, s8]
        nc.vector.tensor_mul(tmpa[:, :], dslice(0), rA())
        nc.vector.tensor_mul(xslice(0), tmpa[:, :], rB())          # +s1 rA rB
        nc.vector.tensor_mul(tmpa[:, :], dslice(1), rA())
        nc.vector.tensor_mul(tmpb[:, :], tmpa[:, :], rN())
        nc.vector.tensor_scalar_mul(xslice(1), tmpb[:, :], -1.0)   # -s2 rA rN
        nc.vector.tensor_copy(xslice(2), dslice(3))                # +s3
        nc.vector.tensor_scalar_mul(xslice(3), dslice(4), -1.0)    # -s4
        nc.vector.tensor_mul(xslice(4), dslice(5), rB())           # +s5 rB
        nc.vector.tensor_mul(tmpa[:, :], dslice(6), rN())
        nc.vector.tensor_scalar_mul(xslice(5), tmpa[:, :], -1.0)   # -s6 rN
        nc.vector.tensor_mul(xslice(6), dslice(2), rA())           # +s7 rA
        nc.vector.tensor_mul(tmpa[:, :], dslice(7), rN())
        nc.vector.tensor_scalar_mul(xslice(7), tmpa[:, :], -1.0)   # -s8 rN

        sg = coll_p.tile([1, 8 * BL], F32, name="sg")
        pl = coll_p.tile([1, 8 * BL], F32, name="pl")
        nc.scalar.activation(sg[:, :], xs[:, :], AF.Sigmoid)
        nc.vector.tensor_scalar_add(sg[:, :], sg[:, :], 0.001)
        nc.scalar.activation(pl[:, :], sg[:, :], AF.Ln)

        def pslice(k):
            return pl[0:1, k::8]

        acc1 = coll_p.tile([1, BL], F32, name="acc1")
        acc3 = coll_p.tile([1, BL], F32, name="acc3")
        nc.vector.tensor_add(acc1[:, :], pslice(0), pslice(1))
        nc.vector.tensor_add(acc3[:, :], pslice(2), pslice(3))
        for k in (4, 5, 6, 7):
            nc.vector.tensor_add(acc3[:, :], acc3[:, :], pslice(k))
        nc.vector.tensor_scalar_mul(acc3[:, :], acc3[:, :], 0.3)
        nc.vector.tensor_add(acc1[:, :], acc1[:, :], acc3[:, :])
        lsum = coll_p.tile([1, 1], F32, name="lsum")
        nc.vector.tensor_reduce(lsum[:, :], acc1[:, :], axis=AXL.X, op=ALU.add)
        nc.vector.tensor_scalar_mul(lsum[:, :], lsum[:, :], -1.0)
        nc.sync.dma_start(out=lossd.ap(), in_=lsum[:, :])


# ----------------------------------------------------------------------------
# host side
# ----------------------------------------------------------------------------

def _text_idx_arrays(T):
    """T: [BL, L] int -> (lo, hi) wrapped int16 [128, NTOK/16]."""
    flat = T.reshape(-1).astype(np.int64)
    lo = np.where(flat < HALF, flat + 1, 0).astype(np.int16)
    hi = np.where(flat >= HALF, flat - HALF + 1, 0).astype(np.int16)
    return _wrap_idx(lo), _wrap_idx(hi)


def _node_idx_arrays(Na, Nb, Nn):
    inter = np.stack([Na, Nb, Nn], axis=1).reshape(-1).astype(np.int64)  # [192]
    inter = np.concatenate([inter, np.full(NIDX - inter.shape[0], -10, np.int64)])
    outs = []
    for lo, hi in NSPL:
        sel = (inter >= lo) & (inter < hi)
        ids = np.where(sel, inter - lo + 1, 0).astype(np.int16)
        outs.append(_wrap_idx(ids))
    return outs


_CACHED_NC = None


def kernel(**inputs):
    global _CACHED_NC
    text_emb = np.asarray(inputs["text_emb"], np.float32)
    node_emb = np.asarray(inputs["node_emb"], np.float32)
    conv_w = np.asarray(inputs["conv_w"], np.float32)
    conv_b = np.asarray(inputs["conv_b"], np.float32)
    rmat = np.asarray(inputs["rand_matrix"], np.float32)

    tlo_a = _pad_rows(text_emb[:HALF])                   # [32768, 128]
    thi_a = _pad_rows(text_emb[HALF:])
    ntab_a = [_pad_rows(node_emb[lo:hi]) for lo, hi in NSPL]
    w0t_a = np.zeros((EP, C), bf16); w0t_a[:E] = conv_w[:, 0, 0, :].T.astype(bf16)
    w1t_a = np.zeros((EP, C), bf16); w1t_a[:E] = conv_w[:, 0, 1, :].T.astype(bf16)
    rmat_a = rmat.astype(bf16)
    bias_a = conv_b.reshape(C, 1).astype(np.float32)
    ones_a = np.ones((128, 128), bf16)
    ident_a = np.eye(128, dtype=bf16)

    if _CACHED_NC is None:
        _CACHED_NC = build_bass()
    nc = _CACHED_NC

    in_maps = []
    for core in range(NCORES):
        sl = slice(core * BL, (core + 1) * BL)
        tA = np.asarray(inputs["Text_a"])[sl]
        tB = np.asarray(inputs["Text_b"])[sl]
        tN = np.asarray(inputs["Text_neg"])[sl]
        nA = np.asarray(inputs["Node_a"])[sl]
        nB = np.asarray(inputs["Node_b"])[sl]
        nN = np.asarray(inputs["Node_neg"])[sl]
        tidx_a = np.stack([w for T in (tA, tB, tN) for w in _text_idx_arrays(T)])
        nidx_a = np.stack(_node_idx_arrays(nA, nB, nN))
        m = {
            "tlo": tlo_a, "thi": thi_a,
            "tidx": tidx_a, "nidx": nidx_a,
            "w0td": w0t_a, "w1td": w1t_a, "rmatd": rmat_a, "biasd": bias_a,
            "onesd": ones_a, "identd": ident_a,
        }
        for k in range(4):
            m[f"ntab{k}"] = ntab_a[k]
        in_maps.append(m)

    res = bass_utils.run_bass_kernel_spmd(nc, in_maps, core_ids=list(range(NCORES)))
    parts = [float(r["loss_out"][0, 0]) for r in res.results]
    return np.float32(np.sum(parts, dtype=np.float64))


# revision 35
# speedup vs baseline: 2.4038x; 1.2577x over previous
"""Trainium2 Bass kernel for nn_CANE: data-parallel over batch on 8 NeuronCores.

Strategy: shard the batch (512 -> 64 items/core). Embedding tables (bf16,
rows padded to 128 elems = 256B) are replicated to every core's DRAM and
gathered on-device via transpose-mode dma_gather (text vocab split into two
<=32768-row halves to fit int16 indices; misses redirect to an all-zero row 0
and the two half-gathers are summed). All matmuls run in bf16 with fp32 PSUM
accumulation. Per-core scalar partial losses are summed on the host.

Pipeline layout (v2): separate PSUM tiles per stage (conv 3 banks, att 3
banks, transpose 1 bank, small 1 bank) so consecutive items overlap across
engines. Attention-weight reductions run on the PE (column sums as N=1
matmuls, conv vectors as transposed-h matvecs); PSUM->SBUF copies run on
the Pool engine; the ones rows / zero pads are memset once outside the loop.
"""

import numpy as np
import ml_dtypes

import concourse.bass as bass
import concourse.bacc as bacc
import concourse.mybir as mybir
from concourse.tile import TileContext
from concourse import bass_utils

bf16 = ml_dtypes.bfloat16
F32 = mybir.dt.float32
BF = mybir.dt.bfloat16
I16 = mybir.dt.int16

B, NCORES = 512, 8
BL = B // NCORES            # 64 items per core
L, LM = 300, 299
E, C, V, NN = 100, 100, 50000, 100000
EP = 128                    # padded embedding row (256B in bf16)
NTOK = BL * L               # 19200 text tokens per tensor per core
TCH = 3200                  # gather chunk (25*128)
NCH = NTOK // TCH
HALF = 32767                # node ids split across 4 tables of <=HALF rows
NIDX = 256                  # node gather size (192 used, padded)
UMAX = 16800                # deduped text-table rows (expected ~16k unique)
AF = mybir.ActivationFunctionType
ALU = mybir.AluOpType
AXL = mybir.AxisListType
# node tables: 4 splits of HALF ids each
NSPL = [(0, HALF), (HALF, 2 * HALF), (2 * HALF, 3 * HALF), (3 * HALF, NN)]
NTAB_R = [hi - lo + 1 for lo, hi in NSPL]

L_CK = [(0, 128), (128, 256), (256, 299)]   # l/m-chunks (the last is 43 wide)


def _wrap_idx(flat):
    """int16 flat index list -> [128, n/16] wrapped (i%16, i//16), x8 replicated."""
    n = flat.shape[0]
    assert n % 16 == 0
    w = flat.reshape(n // 16, 16).T.astype(np.int16)      # [16, n/16]
    return np.tile(w, (8, 1))                              # [128, n/16]


def _pad_rows(tab_f32):
    out = np.zeros((tab_f32.shape[0] + 1, EP), dtype=bf16)
    out[1:, :E] = tab_f32.astype(bf16)
    return out


def build_bass():
    nc = bacc.Bacc("TRN2", target_bir_lowering=False, debug=False)

    ttab = [nc.dram_tensor(f"ttab{t}", [UMAX, EP], BF, kind="ExternalInput")
            for t in range(3)]
    ntab = [nc.dram_tensor(f"ntab{k}", [NTAB_R[k], EP], BF, kind="ExternalInput")
            for k in range(4)]
    tidx = nc.dram_tensor("tidx", [3, 128, NTOK // 16], I16, kind="ExternalInput")
    nidx = nc.dram_tensor("nidx", [4, 128, NIDX // 16], I16, kind="ExternalInput")
    w0td = nc.dram_tensor("w0td", [EP, C], BF, kind="ExternalInput")
    w1td = nc.dram_tensor("w1td", [EP, C], BF, kind="ExternalInput")
    rmatd = nc.dram_tensor("rmatd", [C, C], BF, kind="ExternalInput")
    biasd = nc.dram_tensor("biasd", [C, 1], F32, kind="ExternalInput")
    onesd = nc.dram_tensor("onesd", [128, 128], BF, kind="ExternalInput")  # all-ones
    identd = nc.dram_tensor("identd", [128, 128], BF, kind="ExternalInput")
    lossd = nc.dram_tensor("loss_out", [1, 1], F32, kind="ExternalOutput")

    with TileContext(nc) as tc:
        _emit(nc, tc, ttab, ntab, tidx, nidx, w0td, w1td, rmatd, biasd,
              onesd, identd, lossd)
    nc.compile()  # Bacc: split multi-waits, insert library/act-table loads, lower ISA
    return nc


def _emit(nc, tc, ttab, ntab, tidx, nidx, w0td, w1td, rmatd, biasd,
          onesd, identd, lossd):
    import contextlib
    ctx = contextlib.ExitStack()
    with ctx:
        const_p = ctx.enter_context(tc.tile_pool(name="const", bufs=1))
        txt_p = ctx.enter_context(tc.tile_pool(name="txt", bufs=1))
        raw_p = ctx.enter_context(tc.tile_pool(name="raw", bufs=2))
        work_p = ctx.enter_context(tc.tile_pool(name="work", bufs=1))
        coll_p = ctx.enter_context(tc.tile_pool(name="coll", bufs=1))
        convps_p = ctx.enter_context(tc.tile_pool(name="convps", bufs=1, space="PSUM"))
        attps_p = ctx.enter_context(tc.tile_pool(name="attps", bufs=1, space="PSUM"))
        trxps_p = ctx.enter_context(tc.tile_pool(name="trxps", bufs=1, space="PSUM"))
        hmrps_p = ctx.enter_context(tc.tile_pool(name="hmrps", bufs=1, space="PSUM"))

        # ---- constants into SBUF ----
        w0t = const_p.tile([EP, C], BF, name="w0t")
        w1t = const_p.tile([EP, C], BF, name="w1t")
        rmat = const_p.tile([C, C], BF, name="rmat")
        biasb = const_p.tile([C, 1], F32, name="biasb")
        onesb = const_p.tile([128, 128], BF, name="onesb")
        identb = const_p.tile([128, 128], BF, name="identb")
        nc.sync.dma_start(out=w0t[:, :], in_=w0td.ap())
        nc.sync.dma_start(out=w1t[:, :], in_=w1td.ap())
        nc.sync.dma_start(out=rmat[:, :], in_=rmatd.ap())
        nc.sync.dma_start(out=biasb[:, :], in_=biasd.ap())
        nc.sync.dma_start(out=onesb[:, :], in_=onesd.ap())
        nc.sync.dma_start(out=identb[:, :], in_=identd.ap())

        # ---- index tiles ----
        tix = const_p.tile([128, 3 * (NTOK // 16)], I16, name="tix")
        nix = const_p.tile([128, 4 * (NIDX // 16)], I16, name="nix")
        for t in range(3):
            nc.sync.dma_start(out=tix[:, t * (NTOK // 16):(t + 1) * (NTOK // 16)],
                              in_=tidx.ap()[t])
        for k in range(4):
            nc.sync.dma_start(out=nix[:, k * (NIDX // 16):(k + 1) * (NIDX // 16)],
                              in_=nidx.ap()[k])

        # ---- node gather: 4 splits summed; cols 3b+{0,1,2} = nA,nB,nN ----
        # (gathers issued up front on Pool; the DVE sum happens after the
        # item loop so it doesn't block item work in the DVE stream)
        node_sb = coll_p.tile([128, NIDX], BF, name="node_sb")
        nraws = []
        for k in range(4):
            nraw = raw_p.tile([128, 1, NIDX], BF, name=f"nraw{k}", tag=f"nraw{k % 2}")
            nc.gpsimd.dma_gather(
                out_ap=nraw[:, :, :], in_ap=ntab[k].ap(),
                idxs_ap=nix[:, k * (NIDX // 16):(k + 1) * (NIDX // 16)],
                num_idxs=NIDX, num_idxs_reg=NIDX, elem_size=EP, transpose=True)
            nraws.append(nraw)

        txts = []
        for t, tname in enumerate(("A", "B", "N")):
            txt = txt_p.tile([128, NTOK], BF, name=f"txt{tname}")
            txts.append(txt)

        def emit_gathers(c):
            # deduped tables: one gather per tensor per chunk, no miss pass
            for t in range(3):
                txt = txts[t]
                i0 = t * (NTOK // 16) + c * (TCH // 16)
                dst3 = txt.rearrange("p (k n) -> p k n", n=TCH)[:, c:c + 1, :]
                nc.gpsimd.dma_gather(
                    out_ap=dst3, in_ap=ttab[t].ap(),
                    idxs_ap=tix[:, i0:i0 + TCH // 16],
                    num_idxs=TCH, num_idxs_reg=TCH, elem_size=EP, transpose=True,
                    single_packet=False)

        # ---- per-core collectors ----
        convcols = coll_p.tile([101, 3 * BL], F32, name="convcols")
        rawdots = coll_p.tile([1, 8 * BL], F32, name="rawdots")

        # ---- rotating SBUF work tiles (manual rotation; ones/zero pads are
        #      set once and never overwritten inside the loop) ----
        NROT = 5     # hx/hmrq live 3 items ahead of their readers
        MROT = 3     # tanh/transpose outputs only span one item of slack
        hxs, hmqs, t11s, t13s, htxs, eacs, ecols, wraws = ([] for _ in range(8))
        for r in range(NROT):
            hx = work_p.tile([128, 3, LM], BF, name=f"hx{r}")
            nc.vector.memset(hx[96:128, :, :], 1.0)   # rows 100..127 stay 1.0
            hxs.append(hx)
            hmq = work_p.tile([C, 384], BF, name=f"hmq{r}")
            nc.vector.memset(hmq[:, LM:384], 0.0)
            hmqs.append(hmq)
        for r in range(MROT):
            t11s.append(work_p.tile([128, 3, LM], BF, name=f"t11_{r}"))
            t13s.append(work_p.tile([128, 3, LM], BF, name=f"t13_{r}"))
            htxs.append(work_p.tile([128, 9, 104], BF, name=f"htx{r}"))
            eacs.append(work_p.tile([128, 3], BF, name=f"eac{r}"))
            ecols.append(work_p.tile([128, 6], BF, name=f"ecol{r}"))
            wraws.append(work_p.tile([128, 3], F32, name=f"wraw{r}"))

        # ---- static PSUM tiles (8 banks total); item reuse is serialized by
        #      bank-level WAR deps, which is exactly the pipeline rhythm ----
        convp = convps_p.tile([128, 3, 512], F32, name="convp")
        attp = attps_p.tile([128, 3, 512], F32, name="attp")
        # transpose slots padded to 104 elems (208B) for 4-byte PSUM alignment
        trxp = trxps_p.tile([128, 9, 104], BF, name="trxp")
        hmrb = hmrps_p.tile([128, 512], F32, name="hmrb")
        hmrp = hmrb[:, 0:LM]                  # [.., 299]
        ecolps = (hmrb[:, 304:310], hmrb[:, 312:318])     # rotate even/odd item
        cvps = (hmrb[:, 320:323], hmrb[:, 324:327])
        # dots region: hmrb cols 328:392, 8 items x 8 dots

        # ---- 3-stage software pipeline over items, emitted at fine grain so
        #      each engine's in-order stream has a full period of slack ----
        def s0_conv(b):
            cb = b * L
            for t in range(3):
                nc.tensor.matmul(convp[0:C, t, 0:LM], w0t[:, :],
                                 txts[t][:, cb:cb + LM], start=True, stop=False)
            for t in range(3):
                nc.tensor.matmul(convp[0:C, t, 0:LM], w1t[:, :],
                                 txts[t][:, cb + 1:cb + L], start=False, stop=True)

        def s0_convtanh(b):
            hx = hxs[b % NROT]
            # conv-tanh writes rows 0:100; rows 100:127 stay 1.0 from the memset
            nc.scalar.activation(hx[0:C, :, :], convp[0:C, 0:3, 0:LM], AF.Tanh,
                                 bias=biasb[:, :], scale=1.0)

        def s0_hmr(b):
            hx = hxs[b % NROT]
            nc.tensor.matmul(hmrp[0:C, 0:LM], rmat[:, :], hx[0:C, 0, :],
                             start=True, stop=True)

        def s0_hmrcopy(b):
            hmrq = hmqs[b % NROT]
            nc.vector.tensor_copy(hmrq[:, 0:LM], hmrp[0:C, 0:LM])

        def s0_att1(b):
            hx, hmrq = hxs[b % NROT], hmqs[b % NROT]
            for ck in range(3):
                lhs = hmrq[:, ck * 128:(ck + 1) * 128]
                nc.tensor.matmul(attp[:, ck, 0:LM], lhs, hx[0:C, 1, :],
                                 start=True, stop=True)

        def sa_tanh1(b):
            t11 = t11s[b % MROT]
            nc.scalar.activation(t11[:, :, :], attp[:, 0:3, 0:LM], AF.Tanh)

        def sa_att3(b):
            hx, hmrq = hxs[b % NROT], hmqs[b % NROT]
            for ck in range(3):
                lhs = hmrq[:, ck * 128:(ck + 1) * 128]
                nc.tensor.matmul(attp[:, ck, 0:LM], lhs, hx[0:C, 2, :],
                                 start=True, stop=True)

        def sa_trx(b):
            hx = hxs[b % NROT]
            for t in range(3):
                for ck, (l0, l1) in enumerate(L_CK):
                    w = l1 - l0
                    nc.tensor.transpose(trxp[0:w, 3 * t + ck, 0:101],
                                        hx[0:C + 1, t, l0:l1],
                                        identb[0:C + 1, 0:C + 1])

        def sa_wraw(b):
            nc.vector.tensor_reduce(wraws[b % MROT][:, :],
                                    t11s[b % MROT][:, 0:3, :], axis=AXL.X,
                                    op=ALU.add)

        def sa_tanh3(b):
            t13 = t13s[b % MROT]
            nc.scalar.activation(t13[:, :, :], attp[:, 0:3, 0:LM], AF.Tanh)

        def sa_htx(b):
            nc.vector.tensor_copy(htxs[b % MROT][:, :, :], trxp[:, :, :])

        def sa_eac(b):
            nc.scalar.activation(eacs[b % MROT][:, :], wraws[b % MROT][:, :],
                                 AF.Exp, scale=1.0 / LM)

        def s2_colsum(b):
            r = b % MROT
            ecolp = ecolps[b % 2]
            for j, t1x in enumerate((t11s[r], t13s[r])):
                for mk, (m0, m1) in enumerate(L_CK):
                    for lk, (l0, l1) in enumerate(L_CK):
                        wl = l1 - l0
                        nc.tensor.matmul(
                            ecolp[0:m1 - m0, 3 * j + mk:3 * j + mk + 1],
                            t1x[0:wl, lk, m0:m1], onesb[0:wl, 0:1],
                            start=(lk == 0), stop=(lk == 2))

        def s2_exp(b):
            nc.scalar.activation(ecols[b % MROT][:, :], ecolps[b % 2][:, :],
                                 AF.Exp, scale=1.0 / LM)

        def s2_matvec(b):
            r = b % MROT
            htx, eac, ecol = htxs[r], eacs[r], ecols[r]
            cvp = cvps[b % 2]
            for t in range(3):
                for ck, (l0, l1) in enumerate(L_CK):
                    w = l1 - l0
                    if t == 0:
                        ecv = eac[0:w, ck:ck + 1]
                    else:
                        ecv = ecol[0:w, 3 * (t - 1) + ck:3 * (t - 1) + ck + 1]
                    nc.tensor.matmul(cvp[0:C + 1, t:t + 1],
                                     htx[0:w, 3 * t + ck, 0:101], ecv,
                                     start=(ck == 0), stop=(ck == 2))
            nc.vector.tensor_copy(convcols[:, 3 * b:3 * b + 3], cvp[0:C + 1, :])

        # ---- interleaved emission (4-stage: conv/hmr two items ahead so the
        #      hmr-copy chain never gates tanh1; att1 one item ahead) ----
        emit_gathers(0)
        emit_gathers(1)
        s0_conv(0)
        s0_convtanh(0)
        s0_hmr(0)
        s0_hmrcopy(0)
        gathered = 2
        for b in range(BL):
            c_need = min(NCH - 1, ((min(BL - 1, b + 3) + 1) * L - 1) // TCH)
            while gathered <= min(NCH - 1, c_need + 1):
                emit_gathers(gathered)
                gathered += 1
            if b == 0:
                for j in (1, 2):
                    s0_conv(j)
                    s0_convtanh(j)
                    s0_hmr(j)
                    s0_hmrcopy(j)
                s0_att1(0)
            sa_tanh1(b)
            sa_att3(b)
            sa_tanh3(b)
            sa_trx(b)
            sa_wraw(b)
            if b + 2 < BL:
                s0_hmr(b + 2)
                s0_hmrcopy(b + 2)
            sa_htx(b)
            sa_eac(b)
            if b + 1 < BL:
                s0_att1(b + 1)
            s2_colsum(b)
            if b > 0:
                s2_exp(b - 1)
                s2_matvec(b - 1)
            # conv three items ahead: its tanh is the Act filler that hides
            # the att1-mm latency before the next item's tanh1
            if b + 3 < BL:
                s0_conv(b + 3)
                s0_convtanh(b + 3)
        s2_exp(BL - 1)
        s2_matvec(BL - 1)

        # node embedding sum (deferred so the DVE stream stays clear)
        nc.vector.tensor_copy(node_sb[:, :], nraws[0][:, 0, :])
        for k in (1, 2, 3):
            nc.vector.tensor_add(node_sb[:, :], node_sb[:, :], nraws[k][:, 0, :])

        # ---- dots phase ----
        ccb = coll_p.tile([101, 3 * BL], BF, name="ccb")
        nc.vector.tensor_copy(ccb[:, :], convcols[:, :])
        for b in range(BL):
            g = b % 8
            dps = hmrb[0:1, 328 + 8 * g:328 + 8 * g + 8]
            cA = ccb[0:C, 3 * b:3 * b + 1]
            cBN = ccb[0:C, 3 * b + 1:3 * b + 3]
            nA = node_sb[0:C, 3 * b:3 * b + 1]
            nBc = node_sb[0:C, 3 * b + 1:3 * b + 2]
            nBN = node_sb[0:C, 3 * b + 1:3 * b + 3]
            nc.tensor.matmul(dps[:, 0:2], cA, cBN, start=True, stop=True)
            nc.tensor.matmul(dps[:, 2:3], cA, nBc, start=True, stop=True)
            nc.tensor.matmul(dps[:, 3:5], nA, nBN, start=True, stop=True)
            nc.tensor.matmul(dps[:, 5:7], nA, cBN, start=True, stop=True)
            nc.tensor.matmul(dps[:, 7:8], nBc, ccb[0:C, 3 * b + 2:3 * b + 3],
                             start=True, stop=True)
            if g == 7:
                nc.vector.tensor_copy(rawdots[:, 8 * (b - 7):8 * (b + 1)],
                                      hmrb[0:1, 328:328 + 64])

        # ---- finals (row layout, vectorized over the 64 items) ----
        srow = coll_p.tile([1, 3 * BL], F32, name="srow")
        nc.sync.dma_start(out=srow[:, :], in_=convcols[C:C + 1, :])
        rr = coll_p.tile([1, 3 * BL], F32, name="rr")
        nc.vector.reciprocal(rr[:, :], srow[:, :])
        xs = coll_p.tile([1, 8 * BL], F32, name="xs")
        tmpa = coll_p.tile([1, BL], F32, name="tmpa")
        tmpb = coll_p.tile([1, BL], F32, name="tmpb")

        def dslice(k):
            return rawdots[0:1, k::8]

        def xslice(k):
            return xs[0:1, k::8]

        def rA():
            return rr[0:1, 0::3]

        def rB():
            return rr[0:1, 1::3]

        def rN():
            return rr[0:1, 2::3]

        # rawdots col order: [s1, s2, s7, s3, s4, s5, s6, s8]
        nc.vector.tensor_mul(tmpa[:, :], dslice(0), rA())
        nc.vector.tensor_mul(xslice(0), tmpa[:, :], rB())          # +s1 rA rB
        nc.vector.tensor_mul(tmpa[:, :], dslice(1), rA())
        nc.vector.tensor_mul(tmpb[:, :], tmpa[:, :], rN())
        nc.vector.tensor_scalar_mul(xslice(1), tmpb[:, :], -1.0)   # -s2 rA rN
        nc.vector.tensor_copy(xslice(2), dslice(3))                # +s3
        nc.vector.tensor_scalar_mul(xslice(3), dslice(4), -1.0)    # -s4
        nc.vector.tensor_mul(xslice(4), dslice(5), rB())           # +s5 rB
        nc.vector.tensor_mul(tmpa[:, :], dslice(6), rN())
        nc.vector.tensor_scalar_mul(xslice(5), tmpa[:, :], -1.0)   # -s6 rN
        nc.vector.tensor_mul(xslice(6), dslice(2), rA())           # +s7 rA
        nc.vector.tensor_mul(tmpa[:, :], dslice(7), rN())
        nc.vector.tensor_scalar_mul(xslice(7), tmpa[:, :], -1.0)   # -s8 rN

        sg = coll_p.tile([1, 8 * BL], F32, name="sg")
        pl = coll_p.tile([1, 8 * BL], F32, name="pl")
        nc.scalar.activation(sg[:, :], xs[:, :], AF.Sigmoid)
        nc.vector.tensor_scalar_add(sg[:, :], sg[:, :], 0.001)
        nc.scalar.activation(pl[:, :], sg[:, :], AF.Ln)

        def pslice(k):
            return pl[0:1, k::8]

        acc1 = coll_p.tile([1, BL], F32, name="acc1")
        acc3 = coll_p.tile([1, BL], F32, name="acc3")
        nc.vector.tensor_add(acc1[:, :], pslice(0), pslice(1))
        nc.vector.tensor_add(acc3[:, :], pslice(2), pslice(3))
        for k in (4, 5, 6, 7):
            nc.vector.tensor_add(acc3[:, :], acc3[:, :], pslice(k))
        nc.vector.tensor_scalar_mul(acc3[:, :], acc3[:, :], 0.3)
        nc.vector.tensor_add(acc1[:, :], acc1[:, :], acc3[:, :])
        lsum = coll_p.tile([1, 1], F32, name="lsum")
        nc.vector.tensor_reduce(lsum[:, :], acc1[:, :], axis=AXL.X, op=ALU.add)
        nc.vector.tensor_scalar_mul(lsum[:, :], lsum[:, :], -1.0)
        nc.sync.dma_start(out=lossd.ap(), in_=lsum[:, :])


# ----------------------------------------------------------------------------
# host side
# ----------------------------------------------------------------------------

def _text_dedup(T, text_emb):
    """T: [BL, L] ids -> (wrapped int16 ranks [128, NTOK/16], table [UMAX, EP])."""
    flat = T.reshape(-1).astype(np.int64)
    uniq, inv = np.unique(flat, return_inverse=True)
    assert uniq.size <= UMAX, f"unique vocab {uniq.size} > {UMAX}"
    tab = np.zeros((UMAX, EP), dtype=bf16)
    tab[:uniq.size, :E] = text_emb[uniq].astype(bf16)
    return _wrap_idx(inv.astype(np.int16)), tab


def _node_idx_arrays(Na, Nb, Nn):
    inter = np.stack([Na, Nb, Nn], axis=1).reshape(-1).astype(np.int64)  # [192]
    inter = np.concatenate([inter, np.full(NIDX - inter.shape[0], -10, np.int64)])
    outs = []
    for lo, hi in NSPL:
        sel = (inter >= lo) & (inter < hi)
        ids = np.where(sel, inter - lo + 1, 0).astype(np.int16)
        outs.append(_wrap_idx(ids))
    return outs


_CACHED_NC = None


def kernel(**inputs):
    global _CACHED_NC
    text_emb = np.asarray(inputs["text_emb"], np.float32)
    node_emb = np.asarray(inputs["node_emb"], np.float32)
    conv_w = np.asarray(inputs["conv_w"], np.float32)
    conv_b = np.asarray(inputs["conv_b"], np.float32)
    rmat = np.asarray(inputs["rand_matrix"], np.float32)

    ntab_a = [_pad_rows(node_emb[lo:hi]) for lo, hi in NSPL]
    w0t_a = np.zeros((EP, C), bf16); w0t_a[:E] = conv_w[:, 0, 0, :].T.astype(bf16)
    w1t_a = np.zeros((EP, C), bf16); w1t_a[:E] = conv_w[:, 0, 1, :].T.astype(bf16)
    rmat_a = rmat.astype(bf16)
    bias_a = conv_b.reshape(C, 1).astype(np.float32)
    ones_a = np.ones((128, 128), bf16)
    ident_a = np.eye(128, dtype=bf16)

    if _CACHED_NC is None:
        _CACHED_NC = build_bass()
    nc = _CACHED_NC

    in_maps = []
    for core in range(NCORES):
        sl = slice(core * BL, (core + 1) * BL)
        tA = np.asarray(inputs["Text_a"])[sl]
        tB = np.asarray(inputs["Text_b"])[sl]
        tN = np.asarray(inputs["Text_neg"])[sl]
        nA = np.asarray(inputs["Node_a"])[sl]
        nB = np.asarray(inputs["Node_b"])[sl]
        nN = np.asarray(inputs["Node_neg"])[sl]
        trips = [_text_dedup(T, text_emb) for T in (tA, tB, tN)]
        tidx_a = np.stack([w for w, _ in trips])
        nidx_a = np.stack(_node_idx_arrays(nA, nB, nN))
        m = {
            "tidx": tidx_a, "nidx": nidx_a,
            "w0td": w0t_a, "w1td": w1t_a, "rmatd": rmat_a, "biasd": bias_a,
            "onesd": ones_a, "identd": ident_a,
        }
        for t in range(3):
            m[f"ttab{t}"] = trips[t][1]
        for k in range(4):
            m[f"ntab{k}"] = ntab_a[k]
        in_maps.append(m)

    res = bass_utils.run_bass_kernel_spmd(nc, in_maps, core_ids=list(range(NCORES)))
    parts = [float(r["loss_out"][0, 0]) for r in res.results]
    return np.float32(np.sum(parts, dtype=np.float64))
